# revision 1
# baseline (speedup 1.0000x reference)
"""GRU (r_t=1) Trainium2 kernel: batch-sharded across 8 NeuronCores.

Layout strategy: everything on-device lives transposed ([hidden, time*batch])
so the sequential scan never needs a transpose. Host numpy does all
transposes as part of shard/unshard.
"""

import sys

if "/opt/trn_rl_repo" not in sys.path:
    sys.path.insert(0, "/opt/trn_rl_repo")

from contextlib import ExitStack

import numpy as np

import concourse.bacc as bacc
import concourse.bass as bass
import concourse.mybir as mybir
import concourse.tile as tile
from concourse import bass_utils

NCORES = 8
DIN = 512
DH = 512
CH = DH // 128  # 4 hidden chunks of 128 partitions
AF = mybir.ActivationFunctionType

# dtype knobs (mybir dtypes)
PROJ_DT = mybir.dt.float32  # projection matmul dtype
SCAN_DT = mybir.dt.float32  # recurrence matmul dtype (weights + h state)
SPILL_DT = mybir.dt.float32  # iz/inn DRAM spill dtype


def _np_of(dt):
    return np.dtype(mybir.dt.np(dt))


def build_nc(T: int, BC: int, proj_dt, scan_dt, spill_dt):
    """Build the per-core Bass program. BC = batch per core."""
    R = T * BC  # columns of the transposed activations
    NT1 = R // 512  # phase-1 column chunks
    assert R % 512 == 0
    BLK = 64
    while T % BLK:
        BLK //= 2
    NBLK = T // BLK
    BCOL = BLK * BC  # columns per scan block

    nc = bacc.Bacc("TRN2", target_bir_lowering=False, debug=False)

    seqT = nc.dram_tensor("seqT", [DIN, R], proj_dt, kind="ExternalInput").ap()
    wizT = nc.dram_tensor("wizT", [DIN, DH], proj_dt, kind="ExternalInput").ap()
    winT = nc.dram_tensor("winT", [DIN, DH], proj_dt, kind="ExternalInput").ap()
    whzT = nc.dram_tensor("whzT", [DH, DH], scan_dt, kind="ExternalInput").ap()
    whnT = nc.dram_tensor("whnT", [DH, DH], scan_dt, kind="ExternalInput").ap()
    biasz = nc.dram_tensor("biasz", [128, CH], mybir.dt.float32, kind="ExternalInput").ap()
    biasn = nc.dram_tensor("biasn", [128, CH], mybir.dt.float32, kind="ExternalInput").ap()
    HT = nc.dram_tensor("HT", [CH, 128, R], mybir.dt.float32, kind="ExternalOutput").ap()

    with tile.TileContext(nc) as tc, ExitStack() as ctx:
        const = ctx.enter_context(tc.tile_pool(name="const", bufs=1))
        dram = ctx.enter_context(tc.tile_pool(name="dram", bufs=1, space="DRAM"))

        # spill tensors for the precomputed input projections (transposed)
        izb = dram.tile([CH, 128, R], spill_dt)
        inb = dram.tile([CH, 128, R], spill_dt)

        # weights resident in SBUF: [k-chunk partitions, k index, h_out]
        wiz_sb = const.tile([128, CH, DH], proj_dt)
        win_sb = const.tile([128, CH, DH], proj_dt)
        whz_sb = const.tile([128, CH, DH], scan_dt)
        whn_sb = const.tile([128, CH, DH], scan_dt)
        for sb, dr in ((wiz_sb, wizT), (win_sb, winT), (whz_sb, whzT), (whn_sb, whnT)):
            nc.gpsimd.dma_start(sb[:], dr.rearrange("(c p) h -> p c h", p=128))
        bz_sb = const.tile([128, CH], mybir.dt.float32)
        bn_sb = const.tile([128, CH], mybir.dt.float32)
        nc.gpsimd.dma_start(bz_sb[:], biasz[:])
        nc.gpsimd.dma_start(bn_sb[:], biasn[:])

        # Walrus allows only one sync-wait on a (self-loading) Matmult's
        # LDWEIGHTS half. Touch each freshly-DMA'd tile with a cheap op on
        # the engine that will consume it, so real consumers carry at most
        # one wait.
        junk_ps = ctx.enter_context(tc.tile_pool(name="junkps", bufs=1, space="PSUM"))
        junk = junk_ps.tile([128, 8], mybir.dt.float32)
        scratch = const.tile([128, 8], mybir.dt.float32)

        def pe_touch(ap_k1):
            nc.tensor.matmul(junk[0:1, 0:1], ap_k1, ap_k1, start=True, stop=True)

        for sb in (wiz_sb, win_sb, whz_sb, whn_sb):
            pe_touch(sb[:, 0, 0:1])
        nc.scalar.copy(scratch[0:1, 0:1], bz_sb[0:1, 0:1])
        nc.scalar.copy(scratch[0:1, 1:2], bn_sb[0:1, 1:2])

        # ---- Phase 1: izT/innT = W @ seqT + bias, spilled to DRAM ----
        with (
            tc.tile_pool(name="seqp", bufs=3) as seq_pool,
            tc.tile_pool(name="p1out", bufs=4) as out_pool,
            tc.tile_pool(name="psum1", bufs=2, space="PSUM") as psum1,
        ):
            for n in range(NT1):
                csl = slice(n * 512, (n + 1) * 512)
                sq = seq_pool.tile([128, CH, 512], proj_dt, tag="sq")
                nc.gpsimd.dma_start(sq[:], seqT[:, csl].rearrange("(c p) r -> p c r", p=128))
                pe_touch(sq[:, 0, 0:1])
                for w_sb, b_sb, spill in ((wiz_sb, bz_sb, izb), (win_sb, bn_sb, inb)):
                    for m in range(CH):
                        ps = psum1.tile([128, 512], mybir.dt.float32, tag="ps1")
                        for k in range(CH):
                            nc.tensor.matmul(
                                ps[:],
                                w_sb[:, k, m * 128 : (m + 1) * 128],
                                sq[:, k, :],
                                start=(k == 0),
                                stop=(k == CH - 1),
                            )
                        st = out_pool.tile([128, 512], spill_dt, tag="st")
                        nc.scalar.activation(
                            st[:], ps[:], AF.Identity, bias=b_sb[:, m : m + 1], scale=1.0
                        )
                        nc.gpsimd.dma_start(spill[m, :, csl], st[:])

        # ---- Phase 2: sequential scan over T steps ----
        with (
            tc.tile_pool(name="io2", bufs=2) as io_pool,
            tc.tile_pool(name="ht2", bufs=2) as ht_pool,
            tc.tile_pool(name="st2", bufs=3) as state_pool,
            tc.tile_pool(name="ew2", bufs=2) as ew_pool,
            tc.tile_pool(name="psum2", bufs=2, space="PSUM") as psum2,
        ):
            h = state_pool.tile([128, CH, BC], scan_dt, tag="h")
            nc.vector.memset(h[:], 0.0)

            for blk in range(NBLK):
                bsl = slice(blk * BCOL, (blk + 1) * BCOL)
                izt = io_pool.tile([128, CH, BCOL], spill_dt, tag="izt")
                int_ = io_pool.tile([128, CH, BCOL], spill_dt, tag="int")
                nc.gpsimd.dma_start(izt[:], izb[:, :, bsl].rearrange("c p r -> p c r"))
                nc.gpsimd.dma_start(int_[:], inb[:, :, bsl].rearrange("c p r -> p c r"))
                nc.vector.tensor_copy(scratch[0:1, 2:3], izt[0:1, 0, 0:1])
                nc.vector.tensor_copy(scratch[0:1, 3:4], int_[0:1, 0, 0:1])
                htb = ht_pool.tile([128, CH, BCOL], mybir.dt.float32, tag="htb")

                for tl in range(BLK):
                    tsl = slice(tl * BC, (tl + 1) * BC)
                    psz = psum2.tile([128, CH, BC], mybir.dt.float32, tag="psz")
                    psn = psum2.tile([128, CH, BC], mybir.dt.float32, tag="psn")
                    for w_sb, ps in ((whz_sb, psz), (whn_sb, psn)):
                        for m in range(CH):
                            for k in range(CH):
                                nc.tensor.matmul(
                                    ps[:, m, :],
                                    w_sb[:, k, m * 128 : (m + 1) * 128],
                                    h[:, k, :],
                                    start=(k == 0),
                                    stop=(k == CH - 1),
                                )
                    tz = ew_pool.tile([128, CH, BC], mybir.dt.float32, tag="tz")
                    nc.vector.tensor_add(tz[:], psz[:], izt[:, :, tsl])
                    z = ew_pool.tile([128, CH, BC], mybir.dt.float32, tag="z")
                    nc.scalar.activation(z[:], tz[:], AF.Sigmoid)
                    tn = ew_pool.tile([128, CH, BC], mybir.dt.float32, tag="tn")
                    nc.vector.tensor_add(tn[:], psn[:], int_[:, :, tsl])
                    nt = ew_pool.tile([128, CH, BC], mybir.dt.float32, tag="nt")
                    nc.scalar.activation(nt[:], tn[:], AF.Tanh)
                    # h_new = n + z*(h - n)
                    d = ew_pool.tile([128, CH, BC], mybir.dt.float32, tag="d")
                    nc.vector.tensor_sub(d[:], h[:], nt[:])
                    p = ew_pool.tile([128, CH, BC], mybir.dt.float32, tag="p")
                    nc.vector.tensor_mul(p[:], z[:], d[:])
                    hn = state_pool.tile([128, CH, BC], scan_dt, tag="h")
                    nc.vector.tensor_add(hn[:], nt[:], p[:])
                    nc.gpsimd.tensor_copy(htb[:, :, tsl], hn[:])
                    h = hn

                nc.gpsimd.dma_start(HT[:, :, bsl].rearrange("c p r -> p c r"), htb[:])

    nc.compile()
    return nc


_CACHE: dict = {}


def _get_nc(T, BC, proj_dt, scan_dt, spill_dt):
    key = (T, BC, proj_dt, scan_dt, spill_dt)
    if key not in _CACHE:
        _CACHE[key] = build_nc(T, BC, proj_dt, scan_dt, spill_dt)
    return _CACHE[key]


def kernel(seq, W_iz, b_iz, W_in, b_in, W_hz, b_hz, W_hn, b_hn):
    T, B, _ = seq.shape
    BC = B // NCORES
    nc = _get_nc(T, BC, PROJ_DT, SCAN_DT, SPILL_DT)

    pnp = _np_of(PROJ_DT)
    snp = _np_of(SCAN_DT)
    f32 = np.float32

    wizT = np.ascontiguousarray(W_iz.T.astype(pnp))
    winT = np.ascontiguousarray(W_in.T.astype(pnp))
    whzT = np.ascontiguousarray(W_hz.T.astype(snp))
    whnT = np.ascontiguousarray(W_hn.T.astype(snp))
    biasz = np.ascontiguousarray((b_iz + b_hz).astype(f32).reshape(CH, 128).T)
    biasn = np.ascontiguousarray((b_in + b_hn).astype(f32).reshape(CH, 128).T)

    in_maps = []
    for c in range(NCORES):
        shard = seq[:, c * BC : (c + 1) * BC, :].reshape(T * BC, DIN)
        seqT = np.ascontiguousarray(shard.T.astype(pnp))
        in_maps.append(
            {
                "seqT": seqT,
                "wizT": wizT,
                "winT": winT,
                "whzT": whzT,
                "whnT": whnT,
                "biasz": biasz,
                "biasn": biasn,
            }
        )

    res = bass_utils.run_bass_kernel_spmd(nc, in_maps, list(range(NCORES)))

    out = np.empty((T, B, DH), np.float32)
    for c in range(NCORES):
        HT = res.results[c]["HT"]  # [CH, 128, T*BC]
        Hc = HT.reshape(CH, 128, T, BC).transpose(2, 3, 0, 1).reshape(T, BC, DH)
        out[:, c * BC : (c + 1) * BC, :] = Hc
    return out[None]



# revision 2
# speedup vs baseline: 1.0066x; 1.0066x over previous
"""GRU (r_t=1) Trainium2 kernel v4.

vs v3:
- every elementwise tensor is a flat contiguous [128, 64] 2D AP (DVE 2x
  bf16 mode eligibility + less AP overhead)
- iz/inn and the output block are time-major, so per-step slices are
  contiguous and h_new is written directly into the output block by the
  final DVE add (Pool copy eliminated)
- output HT is [128, T, CH*BC] per core; host reassembles
"""

import sys

if "/opt/trn_rl_repo" not in sys.path:
    sys.path.insert(0, "/opt/trn_rl_repo")

from contextlib import ExitStack

import ml_dtypes
import numpy as np

import concourse.bacc as bacc
import concourse.mybir as mybir
import concourse.tile as tile
from concourse import bass_utils

NCORES = 8
DIN = 512
DH = 512
CH = DH // 128
AF = mybir.ActivationFunctionType
BF16 = mybir.dt.bfloat16
F32 = mybir.dt.float32
NPBF16 = np.dtype(ml_dtypes.bfloat16)


def build_nc(T: int, BC: int):
    R = T * BC
    D = CH * BC  # flat per-step element count per partition (64)
    PCOL = 512
    assert R % PCOL == 0
    NT1 = R // PCOL
    CSTEPS = PCOL // BC  # scan steps covered by one phase-1 chunk (32)
    BLK = CSTEPS
    assert T % BLK == 0
    NBLK = T // BLK
    PRO = min(2, NT1)  # prologue chunks

    nc = bacc.Bacc("TRN2", target_bir_lowering=False, debug=False)

    seqT = nc.dram_tensor("seqT", [DIN, R], BF16, kind="ExternalInput").ap()
    wizT = nc.dram_tensor("wizT", [DIN, DH], BF16, kind="ExternalInput").ap()
    winT = nc.dram_tensor("winT", [DIN, DH], BF16, kind="ExternalInput").ap()
    whzT = nc.dram_tensor("whzT", [DH, DH], BF16, kind="ExternalInput").ap()
    whnT = nc.dram_tensor("whnT", [DH, DH], BF16, kind="ExternalInput").ap()
    biasz = nc.dram_tensor("biasz", [128, CH], F32, kind="ExternalInput").ap()
    biasn = nc.dram_tensor("biasn", [128, CH], F32, kind="ExternalInput").ap()
    # time-major flat output: HT[p, t, m*BC+b] ; h[m*128+p] at (t, b)
    HT = nc.dram_tensor("HT", [128, T, D], BF16, kind="ExternalOutput").ap()

    with tile.TileContext(nc) as tc, ExitStack() as ctx:
        const = ctx.enter_context(tc.tile_pool(name="const", bufs=1))

        # iz/inn time-major: [128, t, m*BC+b]
        iz_sb = const.tile([128, T, D], BF16)
        in_sb = const.tile([128, T, D], BF16)

        wiz_sb = const.tile([128, CH, DH], BF16)
        win_sb = const.tile([128, CH, DH], BF16)
        whz_sb = const.tile([128, CH, DH], BF16)
        whn_sb = const.tile([128, CH, DH], BF16)
        for sb, dr in ((wiz_sb, wizT), (win_sb, winT), (whz_sb, whzT), (whn_sb, whnT)):
            nc.gpsimd.dma_start(sb[:], dr.rearrange("(c p) h -> p c h", p=128))
        bz_sb = const.tile([128, CH], F32)
        bn_sb = const.tile([128, CH], F32)
        nc.gpsimd.dma_start(bz_sb[:], biasz[:])
        nc.gpsimd.dma_start(bn_sb[:], biasn[:])

        junk_ps = ctx.enter_context(tc.tile_pool(name="junkps", bufs=1, space="PSUM"))
        junk = junk_ps.tile([128, 8], F32)
        scratch = const.tile([128, 8], F32)

        def pe_touch(ap_k1):
            nc.tensor.matmul(junk[0:1, 0:1], ap_k1, ap_k1, start=True, stop=True)

        for sb in (wiz_sb, win_sb, whz_sb, whn_sb):
            pe_touch(sb[:, 0, 0:1])
        nc.vector.tensor_copy(scratch[0:1, 0:1], bz_sb[0:1, 0:1])
        nc.vector.tensor_copy(scratch[0:1, 1:2], bn_sb[0:1, 1:2])

        seq_pool = ctx.enter_context(tc.tile_pool(name="seqp", bufs=3))
        psum1 = ctx.enter_context(tc.tile_pool(name="psum1", bufs=2, space="PSUM"))

        def chunk_dma(c):
            csl = slice(c * PCOL, (c + 1) * PCOL)
            sq = seq_pool.tile([128, CH, PCOL], BF16, tag="sq")
            nc.gpsimd.dma_start(sq[:], seqT[:, csl].rearrange("(c p) r -> p c r", p=128))
            pe_touch(sq[:, 0, 0:1])
            return sq

        def p1_group_mm(sq, ps, g, k):
            m = g % CH
            w_sb = wiz_sb if g < CH else win_sb
            nc.tensor.matmul(
                ps[:],
                w_sb[:, k, m * 128 : (m + 1) * 128],
                sq[:, k, :],
                start=(k == 0),
                stop=(k == CH - 1),
            )

        def p1_group_bias(ps, c, g):
            t0 = c * CSTEPS
            m = g % CH
            b_sb = bz_sb if g < CH else bn_sb
            dst = iz_sb if g < CH else in_sb
            half = CSTEPS // 2
            for hh in range(2):
                nc.vector.tensor_scalar_add(
                    dst[:, t0 + hh * half : t0 + (hh + 1) * half, m * BC : (m + 1) * BC],
                    ps[:, hh * half * BC : (hh + 1) * half * BC],
                    b_sb[:, m : m + 1],
                )

        for c in range(PRO):
            sq = chunk_dma(c)
            for g in range(2 * CH):
                ps = psum1.tile([128, PCOL], F32, tag="ps1")
                for k in range(CH):
                    p1_group_mm(sq, ps, g, k)
                p1_group_bias(ps, c, g)

        with (
            tc.tile_pool(name="ht2", bufs=2) as ht_pool,
            tc.tile_pool(name="st2", bufs=2) as state_pool,
            tc.tile_pool(name="ew2", bufs=2) as ew_pool,
            tc.tile_pool(name="psum2", bufs=2, space="PSUM") as psum2,
        ):
            h0 = state_pool.tile([128, D], BF16, tag="h0")
            nc.vector.memset(h0[:], 0.0)
            h = h0

            p1_sq = None
            p1_ps = None

            for blk in range(NBLK):
                htb = ht_pool.tile([128, BLK, D], BF16, tag="htb")
                pipe_c = blk + PRO
                if pipe_c < NT1:
                    p1_sq = chunk_dma(pipe_c)

                for tl in range(BLK):
                    t = blk * BLK + tl
                    psz = psum2.tile([128, D], F32, tag="psz")
                    psn = psum2.tile([128, D], F32, tag="psn")
                    for w_sb, ps in ((whz_sb, psz), (whn_sb, psn)):
                        for m in range(CH):
                            for k in range(CH):
                                nc.tensor.matmul(
                                    ps[:, m * BC : (m + 1) * BC],
                                    w_sb[:, k, m * 128 : (m + 1) * 128],
                                    h[:, k * BC : (k + 1) * BC],
                                    start=(k == 0),
                                    stop=(k == CH - 1),
                                )
                    if pipe_c < NT1 and tl < 8 * CH:
                        g, k = tl // CH, tl % CH
                        if k == 0:
                            p1_ps = psum1.tile([128, PCOL], F32, tag="ps1")
                        p1_group_mm(p1_sq, p1_ps, g, k)

                    tz = ew_pool.tile([128, D], F32, tag="tz")
                    nc.vector.tensor_add(tz[:], psz[:], iz_sb[:, t, :])
                    z = ew_pool.tile([128, D], BF16, tag="z")
                    nc.scalar.activation(z[:], tz[:], AF.Sigmoid)
                    zc = ew_pool.tile([128, D], BF16, tag="zc")
                    nc.scalar.activation(zc[:], tz[:], AF.Sigmoid, scale=-1.0)
                    t1 = ew_pool.tile([128, D], BF16, tag="t1")
                    nc.vector.tensor_mul(t1[:], z[:], h[:])
                    tn = ew_pool.tile([128, D], F32, tag="tn")
                    nc.vector.tensor_add(tn[:], psn[:], in_sb[:, t, :])
                    nt = ew_pool.tile([128, D], BF16, tag="nt")
                    nc.scalar.activation(nt[:], tn[:], AF.Tanh)
                    t2 = ew_pool.tile([128, D], BF16, tag="t2")
                    nc.vector.tensor_mul(t2[:], zc[:], nt[:])
                    hn = htb[:, tl, :]
                    nc.vector.tensor_add(hn, t1[:], t2[:])
                    if pipe_c < NT1 and tl < 8 * CH and tl % CH == CH - 1:
                        p1_group_bias(p1_ps, pipe_c, tl // CH)
                    h = hn

                nc.gpsimd.dma_start(HT[:, blk * BLK : (blk + 1) * BLK, :], htb[:])

    nc.compile()
    return nc


_CACHE: dict = {}


def _get_nc(T, BC):
    key = (T, BC)
    if key not in _CACHE:
        _CACHE[key] = build_nc(T, BC)
    return _CACHE[key]


def _in_maps(inputs, T, BC):
    f32 = np.float32
    wizT = np.ascontiguousarray(np.asarray(inputs["W_iz"], f32).T.astype(NPBF16))
    winT = np.ascontiguousarray(np.asarray(inputs["W_in"], f32).T.astype(NPBF16))
    whzT = np.ascontiguousarray(np.asarray(inputs["W_hz"], f32).T.astype(NPBF16))
    whnT = np.ascontiguousarray(np.asarray(inputs["W_hn"], f32).T.astype(NPBF16))
    biasz = np.ascontiguousarray(
        (np.asarray(inputs["b_iz"], f32) + np.asarray(inputs["b_hz"], f32)).reshape(CH, 128).T
    )
    biasn = np.ascontiguousarray(
        (np.asarray(inputs["b_in"], f32) + np.asarray(inputs["b_hn"], f32)).reshape(CH, 128).T
    )
    seq = np.asarray(inputs["seq"], f32)
    in_maps = []
    for c in range(NCORES):
        shard = seq[:, c * BC : (c + 1) * BC, :].reshape(T * BC, DIN)
        seqT = np.ascontiguousarray(shard.T.astype(NPBF16))
        in_maps.append(
            {
                "seqT": seqT,
                "wizT": wizT,
                "winT": winT,
                "whzT": whzT,
                "whnT": whnT,
                "biasz": biasz,
                "biasn": biasn,
            }
        )
    return in_maps


def _assemble(results, T, BC):
    out = np.empty((T, NCORES * BC, DH), np.float32)
    for c in range(NCORES):
        HT = np.asarray(results[c]["HT"]).astype(np.float32)  # [128, T, CH*BC]
        # HT[p, t, m*BC+b] -> out[t, b, m*128+p]
        Hc = HT.reshape(128, T, CH, BC).transpose(1, 3, 2, 0).reshape(T, BC, DH)
        out[:, c * BC : (c + 1) * BC, :] = Hc
    return out[None]


def kernel(seq, W_iz, b_iz, W_in, b_in, W_hz, b_hz, W_hn, b_hn):
    seq = np.asarray(seq)
    T, B, _ = seq.shape
    BC = B // NCORES
    nc = _get_nc(T, BC)
    in_maps = _in_maps(
        dict(seq=seq, W_iz=W_iz, b_iz=b_iz, W_in=W_in, b_in=b_in,
             W_hz=W_hz, b_hz=b_hz, W_hn=W_hn, b_hn=b_hn),
        T, BC,
    )
    res = bass_utils.run_bass_kernel_spmd(nc, in_maps, list(range(NCORES)))
    return _assemble(res.results, T, BC)


def traced_run(inputs):
    seq = np.asarray(inputs["seq"])
    T, B, _ = seq.shape
    BC = B // NCORES
    nc = _get_nc(T, BC)
    in_maps = _in_maps(inputs, T, BC)
    return bass_utils.run_bass_kernel_spmd(
        nc, in_maps, list(range(NCORES)), trace=True
    )


# revision 3
# speedup vs baseline: 1.1250x; 1.1176x over previous
"""GRU (r_t=1) Trainium2 kernel v5.

vs v4:
- h_new = h + (1-z)*(n - h) with (1-z) = sigmoid(-tz): drops the z
  sigmoid and z*h mul entirely (one ACT + one DVE op fewer)
- n-gate matmuls run FIRST in each step's burst so tanh and d = n - h
  complete during the z-gate half; the post-burst tail is just
  tz -> zc -> d2 -> hn (4 ops)
"""

import sys

if "/opt/trn_rl_repo" not in sys.path:
    sys.path.insert(0, "/opt/trn_rl_repo")

from contextlib import ExitStack

import ml_dtypes
import numpy as np

import concourse.bacc as bacc
import concourse.mybir as mybir
import concourse.tile as tile
from concourse import bass_utils

NCORES = 8
DIN = 512
DH = 512
CH = DH // 128
AF = mybir.ActivationFunctionType
BF16 = mybir.dt.bfloat16
F32 = mybir.dt.float32
NPBF16 = np.dtype(ml_dtypes.bfloat16)


def build_nc(T: int, BC: int):
    R = T * BC
    D = CH * BC  # flat per-step element count per partition (64)
    PCOL = 512
    assert R % PCOL == 0
    NT1 = R // PCOL
    CSTEPS = PCOL // BC  # scan steps covered by one phase-1 chunk (32)
    BLK = CSTEPS
    assert T % BLK == 0
    NBLK = T // BLK
    PRO = min(2, NT1)  # prologue chunks

    nc = bacc.Bacc("TRN2", target_bir_lowering=False, debug=False)

    seqT = nc.dram_tensor("seqT", [DIN, R], BF16, kind="ExternalInput").ap()
    wizT = nc.dram_tensor("wizT", [DIN, DH], BF16, kind="ExternalInput").ap()
    winT = nc.dram_tensor("winT", [DIN, DH], BF16, kind="ExternalInput").ap()
    whzT = nc.dram_tensor("whzT", [DH, DH], BF16, kind="ExternalInput").ap()
    whnT = nc.dram_tensor("whnT", [DH, DH], BF16, kind="ExternalInput").ap()
    biasz = nc.dram_tensor("biasz", [128, CH], F32, kind="ExternalInput").ap()
    biasn = nc.dram_tensor("biasn", [128, CH], F32, kind="ExternalInput").ap()
    # time-major flat output: HT[p, t, m*BC+b] ; h[m*128+p] at (t, b)
    HT = nc.dram_tensor("HT", [128, T, D], BF16, kind="ExternalOutput").ap()

    with tile.TileContext(nc) as tc, ExitStack() as ctx:
        const = ctx.enter_context(tc.tile_pool(name="const", bufs=1))

        # iz/inn time-major: [128, t, m*BC+b]
        iz_sb = const.tile([128, T, D], BF16)
        in_sb = const.tile([128, T, D], BF16)

        wiz_sb = const.tile([128, CH, DH], BF16)
        win_sb = const.tile([128, CH, DH], BF16)
        whz_sb = const.tile([128, CH, DH], BF16)
        whn_sb = const.tile([128, CH, DH], BF16)
        for sb, dr in ((wiz_sb, wizT), (win_sb, winT), (whz_sb, whzT), (whn_sb, whnT)):
            nc.gpsimd.dma_start(sb[:], dr.rearrange("(c p) h -> p c h", p=128))
        bz_sb = const.tile([128, CH], F32)
        bn_sb = const.tile([128, CH], F32)
        nc.gpsimd.dma_start(bz_sb[:], biasz[:])
        nc.gpsimd.dma_start(bn_sb[:], biasn[:])

        junk_ps = ctx.enter_context(tc.tile_pool(name="junkps", bufs=1, space="PSUM"))
        junk = junk_ps.tile([128, 8], F32)
        scratch = const.tile([128, 8], F32)

        def pe_touch(ap_k1):
            nc.tensor.matmul(junk[0:1, 0:1], ap_k1, ap_k1, start=True, stop=True)

        for sb in (wiz_sb, win_sb, whz_sb, whn_sb):
            pe_touch(sb[:, 0, 0:1])
        nc.vector.tensor_copy(scratch[0:1, 0:1], bz_sb[0:1, 0:1])
        nc.vector.tensor_copy(scratch[0:1, 1:2], bn_sb[0:1, 1:2])

        seq_pool = ctx.enter_context(tc.tile_pool(name="seqp", bufs=3))
        psum1 = ctx.enter_context(tc.tile_pool(name="psum1", bufs=2, space="PSUM"))

        def chunk_dma(c):
            csl = slice(c * PCOL, (c + 1) * PCOL)
            sq = seq_pool.tile([128, CH, PCOL], BF16, tag="sq")
            nc.gpsimd.dma_start(sq[:], seqT[:, csl].rearrange("(c p) r -> p c r", p=128))
            pe_touch(sq[:, 0, 0:1])
            return sq

        def p1_group_mm(sq, ps, g, k):
            m = g % CH
            w_sb = wiz_sb if g < CH else win_sb
            nc.tensor.matmul(
                ps[:],
                w_sb[:, k, m * 128 : (m + 1) * 128],
                sq[:, k, :],
                start=(k == 0),
                stop=(k == CH - 1),
            )

        def p1_group_bias(ps, c, g):
            t0 = c * CSTEPS
            m = g % CH
            b_sb = bz_sb if g < CH else bn_sb
            dst = iz_sb if g < CH else in_sb
            half = CSTEPS // 2
            for hh in range(2):
                nc.vector.tensor_scalar_add(
                    dst[:, t0 + hh * half : t0 + (hh + 1) * half, m * BC : (m + 1) * BC],
                    ps[:, hh * half * BC : (hh + 1) * half * BC],
                    b_sb[:, m : m + 1],
                )

        for c in range(PRO):
            sq = chunk_dma(c)
            for g in range(2 * CH):
                ps = psum1.tile([128, PCOL], F32, tag="ps1")
                for k in range(CH):
                    p1_group_mm(sq, ps, g, k)
                p1_group_bias(ps, c, g)

        with (
            tc.tile_pool(name="ht2", bufs=2) as ht_pool,
            tc.tile_pool(name="st2", bufs=2) as state_pool,
            tc.tile_pool(name="ew2", bufs=2) as ew_pool,
            tc.tile_pool(name="psum2", bufs=2, space="PSUM") as psum2,
        ):
            h0 = state_pool.tile([128, D], BF16, tag="h0")
            nc.vector.memset(h0[:], 0.0)
            h = h0

            p1_sq = None
            p1_ps = None

            for blk in range(NBLK):
                htb = ht_pool.tile([128, BLK, D], BF16, tag="htb")
                pipe_c = blk + PRO
                if pipe_c < NT1:
                    p1_sq = chunk_dma(pipe_c)

                for tl in range(BLK):
                    t = blk * BLK + tl
                    psz = psum2.tile([128, D], F32, tag="psz")
                    psn = psum2.tile([128, D], F32, tag="psn")
                    for w_sb, ps in ((whn_sb, psn), (whz_sb, psz)):
                        for m in range(CH):
                            for k in range(CH):
                                nc.tensor.matmul(
                                    ps[:, m * BC : (m + 1) * BC],
                                    w_sb[:, k, m * 128 : (m + 1) * 128],
                                    h[:, k * BC : (k + 1) * BC],
                                    start=(k == 0),
                                    stop=(k == CH - 1),
                                )
                    if pipe_c < NT1 and tl < 8 * CH:
                        g, k = tl // CH, tl % CH
                        if k == 0:
                            p1_ps = psum1.tile([128, PCOL], F32, tag="ps1")
                        p1_group_mm(p1_sq, p1_ps, g, k)

                    # n path first: tn/tanh/d run while the z-gate matmuls finish
                    tn = ew_pool.tile([128, D], F32, tag="tn")
                    nc.vector.tensor_add(tn[:], psn[:], in_sb[:, t, :])
                    nt = ew_pool.tile([128, D], BF16, tag="nt")
                    nc.scalar.activation(nt[:], tn[:], AF.Tanh)
                    d = ew_pool.tile([128, D], BF16, tag="d")
                    nc.vector.tensor_sub(d[:], nt[:], h[:])
                    # z path: h_new = h + sigmoid(-tz)*(n - h)
                    tz = ew_pool.tile([128, D], F32, tag="tz")
                    nc.vector.tensor_add(tz[:], psz[:], iz_sb[:, t, :])
                    zc = ew_pool.tile([128, D], BF16, tag="zc")
                    nc.scalar.activation(zc[:], tz[:], AF.Sigmoid, scale=-1.0)
                    d2 = ew_pool.tile([128, D], BF16, tag="d2")
                    nc.vector.tensor_mul(d2[:], zc[:], d[:])
                    hn = htb[:, tl, :]
                    nc.vector.tensor_add(hn, h[:], d2[:])
                    if pipe_c < NT1 and tl < 8 * CH and tl % CH == CH - 1:
                        p1_group_bias(p1_ps, pipe_c, tl // CH)
                    h = hn

                nc.gpsimd.dma_start(HT[:, blk * BLK : (blk + 1) * BLK, :], htb[:])

    nc.compile()
    return nc


_CACHE: dict = {}


def _get_nc(T, BC):
    key = (T, BC)
    if key not in _CACHE:
        _CACHE[key] = build_nc(T, BC)
    return _CACHE[key]


def _in_maps(inputs, T, BC):
    f32 = np.float32
    wizT = np.ascontiguousarray(np.asarray(inputs["W_iz"], f32).T.astype(NPBF16))
    winT = np.ascontiguousarray(np.asarray(inputs["W_in"], f32).T.astype(NPBF16))
    whzT = np.ascontiguousarray(np.asarray(inputs["W_hz"], f32).T.astype(NPBF16))
    whnT = np.ascontiguousarray(np.asarray(inputs["W_hn"], f32).T.astype(NPBF16))
    biasz = np.ascontiguousarray(
        (np.asarray(inputs["b_iz"], f32) + np.asarray(inputs["b_hz"], f32)).reshape(CH, 128).T
    )
    biasn = np.ascontiguousarray(
        (np.asarray(inputs["b_in"], f32) + np.asarray(inputs["b_hn"], f32)).reshape(CH, 128).T
    )
    seq = np.asarray(inputs["seq"], f32)
    in_maps = []
    for c in range(NCORES):
        shard = seq[:, c * BC : (c + 1) * BC, :].reshape(T * BC, DIN)
        seqT = np.ascontiguousarray(shard.T.astype(NPBF16))
        in_maps.append(
            {
                "seqT": seqT,
                "wizT": wizT,
                "winT": winT,
                "whzT": whzT,
                "whnT": whnT,
                "biasz": biasz,
                "biasn": biasn,
            }
        )
    return in_maps


def _assemble(results, T, BC):
    out = np.empty((T, NCORES * BC, DH), np.float32)
    for c in range(NCORES):
        HT = np.asarray(results[c]["HT"]).astype(np.float32)  # [128, T, CH*BC]
        # HT[p, t, m*BC+b] -> out[t, b, m*128+p]
        Hc = HT.reshape(128, T, CH, BC).transpose(1, 3, 2, 0).reshape(T, BC, DH)
        out[:, c * BC : (c + 1) * BC, :] = Hc
    return out[None]


def kernel(seq, W_iz, b_iz, W_in, b_in, W_hz, b_hz, W_hn, b_hn):
    seq = np.asarray(seq)
    T, B, _ = seq.shape
    BC = B // NCORES
    nc = _get_nc(T, BC)
    in_maps = _in_maps(
        dict(seq=seq, W_iz=W_iz, b_iz=b_iz, W_in=W_in, b_in=b_in,
             W_hz=W_hz, b_hz=b_hz, W_hn=W_hn, b_hn=b_hn),
        T, BC,
    )
    res = bass_utils.run_bass_kernel_spmd(nc, in_maps, list(range(NCORES)))
    return _assemble(res.results, T, BC)


def traced_run(inputs):
    seq = np.asarray(inputs["seq"])
    T, B, _ = seq.shape
    BC = B // NCORES
    nc = _get_nc(T, BC)
    in_maps = _in_maps(inputs, T, BC)
    return bass_utils.run_bass_kernel_spmd(
        nc, in_maps, list(range(NCORES)), trace=True
    )


# revision 4
# speedup vs baseline: 1.1291x; 1.0037x over previous
"""GRU (r_t=1) Trainium2 kernel v6.

vs v5:
- persistent PSUM accumulators: since h_t = h_{t-1} + d2_t and matmul is
  linear, ps_{t+1} = ps_t + W @ d2_t. The scan matmuls take d2 (not h_new)
  as the moving operand, so the PE restart is gated by d2 and the final
  h-add drops off the critical path.
- h state carried in fp32 (output block) so the elementwise state equals
  the PSUM-implied sum exactly (no drift); d2 stays bf16 for the PE.
- tz/tn live in PSUM: sigmoid/tanh read via ACT's faster PSUM port.
"""

import sys

if "/opt/trn_rl_repo" not in sys.path:
    sys.path.insert(0, "/opt/trn_rl_repo")

from contextlib import ExitStack

import ml_dtypes
import numpy as np

import concourse.bacc as bacc
import concourse.mybir as mybir
import concourse.tile as tile
from concourse import bass_utils

NCORES = 8
DIN = 512
DH = 512
CH = DH // 128
AF = mybir.ActivationFunctionType
BF16 = mybir.dt.bfloat16
F32 = mybir.dt.float32
NPBF16 = np.dtype(ml_dtypes.bfloat16)


def build_nc(T: int, BC: int):
    R = T * BC
    D = CH * BC  # flat per-step element count per partition (64)
    PCOL = 512
    assert R % PCOL == 0
    NT1 = R // PCOL
    CSTEPS = PCOL // BC  # scan steps covered by one phase-1 chunk (32)
    BLK = CSTEPS
    assert T % BLK == 0
    NBLK = T // BLK
    PRO = min(2, NT1)  # prologue chunks

    nc = bacc.Bacc("TRN2", target_bir_lowering=False, debug=False)

    seqT = nc.dram_tensor("seqT", [DIN, R], BF16, kind="ExternalInput").ap()
    wizT = nc.dram_tensor("wizT", [DIN, DH], BF16, kind="ExternalInput").ap()
    winT = nc.dram_tensor("winT", [DIN, DH], BF16, kind="ExternalInput").ap()
    whzT = nc.dram_tensor("whzT", [DH, DH], BF16, kind="ExternalInput").ap()
    whnT = nc.dram_tensor("whnT", [DH, DH], BF16, kind="ExternalInput").ap()
    biasz = nc.dram_tensor("biasz", [128, CH], F32, kind="ExternalInput").ap()
    biasn = nc.dram_tensor("biasn", [128, CH], F32, kind="ExternalInput").ap()
    # time-major flat output: HT[p, t, m*BC+b] ; h[m*128+p] at (t, b)
    HT = nc.dram_tensor("HT", [128, T, D], F32, kind="ExternalOutput").ap()

    with tile.TileContext(nc) as tc, ExitStack() as ctx:
        const = ctx.enter_context(tc.tile_pool(name="const", bufs=1))

        # iz/inn time-major: [128, t, m*BC+b]
        iz_sb = const.tile([128, T, D], BF16)
        in_sb = const.tile([128, T, D], BF16)

        wiz_sb = const.tile([128, CH, DH], BF16)
        win_sb = const.tile([128, CH, DH], BF16)
        whz_sb = const.tile([128, CH, DH], BF16)
        whn_sb = const.tile([128, CH, DH], BF16)
        for sb, dr in ((wiz_sb, wizT), (win_sb, winT), (whz_sb, whzT), (whn_sb, whnT)):
            nc.gpsimd.dma_start(sb[:], dr.rearrange("(c p) h -> p c h", p=128))
        bz_sb = const.tile([128, CH], F32)
        bn_sb = const.tile([128, CH], F32)
        nc.gpsimd.dma_start(bz_sb[:], biasz[:])
        nc.gpsimd.dma_start(bn_sb[:], biasn[:])

        junk_ps = ctx.enter_context(tc.tile_pool(name="junkps", bufs=1, space="PSUM"))
        junk = junk_ps.tile([128, 8], F32)
        scratch = const.tile([128, 8], F32)

        def pe_touch(ap_k1):
            nc.tensor.matmul(junk[0:1, 0:1], ap_k1, ap_k1, start=True, stop=True)

        for sb in (wiz_sb, win_sb, whz_sb, whn_sb):
            pe_touch(sb[:, 0, 0:1])
        nc.vector.tensor_copy(scratch[0:1, 0:1], bz_sb[0:1, 0:1])
        nc.vector.tensor_copy(scratch[0:1, 1:2], bn_sb[0:1, 1:2])

        seq_pool = ctx.enter_context(tc.tile_pool(name="seqp", bufs=3))
        psum1 = ctx.enter_context(tc.tile_pool(name="psum1", bufs=2, space="PSUM"))

        def chunk_dma(c):
            csl = slice(c * PCOL, (c + 1) * PCOL)
            sq = seq_pool.tile([128, CH, PCOL], BF16, tag="sq")
            nc.gpsimd.dma_start(sq[:], seqT[:, csl].rearrange("(c p) r -> p c r", p=128))
            pe_touch(sq[:, 0, 0:1])
            return sq

        def p1_group_mm(sq, ps, g, k):
            m = g % CH
            w_sb = wiz_sb if g < CH else win_sb
            nc.tensor.matmul(
                ps[:],
                w_sb[:, k, m * 128 : (m + 1) * 128],
                sq[:, k, :],
                start=(k == 0),
                stop=(k == CH - 1),
            )

        def p1_group_bias(ps, c, g):
            t0 = c * CSTEPS
            m = g % CH
            b_sb = bz_sb if g < CH else bn_sb
            dst = iz_sb if g < CH else in_sb
            half = CSTEPS // 2
            for hh in range(2):
                nc.vector.tensor_scalar_add(
                    dst[:, t0 + hh * half : t0 + (hh + 1) * half, m * BC : (m + 1) * BC],
                    ps[:, hh * half * BC : (hh + 1) * half * BC],
                    b_sb[:, m : m + 1],
                )

        for c in range(PRO):
            sq = chunk_dma(c)
            for g in range(2 * CH):
                ps = psum1.tile([128, PCOL], F32, tag="ps1")
                for k in range(CH):
                    p1_group_mm(sq, ps, g, k)
                p1_group_bias(ps, c, g)

        with (
            tc.tile_pool(name="ht2", bufs=2) as ht_pool,
            tc.tile_pool(name="st2", bufs=2) as state_pool,
            tc.tile_pool(name="ew2", bufs=2) as ew_pool,
            tc.tile_pool(name="accp", bufs=1, space="PSUM") as accp,
            tc.tile_pool(name="ewps", bufs=1, space="PSUM") as ewps,
        ):
            # zero bf16 tile: moving operand for the step-0 seed matmuls
            z0 = state_pool.tile([128, D], BF16, tag="z0")
            nc.vector.memset(z0[:], 0.0)
            h0 = state_pool.tile([128, D], F32, tag="h0")
            nc.vector.memset(h0[:], 0.0)
            h = h0
            d2_prev = z0

            # persistent cross-step accumulators: ps_t = W @ h_{t-1}
            psn_acc = accp.tile([128, D], F32, tag="psn_acc")
            psz_acc = accp.tile([128, D], F32, tag="psz_acc")

            p1_sq = None
            p1_ps = None

            for blk in range(NBLK):
                htb = ht_pool.tile([128, BLK, D], F32, tag="htb")
                pipe_c = blk + PRO
                if pipe_c < NT1:
                    p1_sq = chunk_dma(pipe_c)

                for tl in range(BLK):
                    t = blk * BLK + tl
                    for w_sb, ps in ((whn_sb, psn_acc), (whz_sb, psz_acc)):
                        for m in range(CH):
                            for k in range(CH):
                                nc.tensor.matmul(
                                    ps[:, m * BC : (m + 1) * BC],
                                    w_sb[:, k, m * 128 : (m + 1) * 128],
                                    d2_prev[:, k * BC : (k + 1) * BC],
                                    start=(t == 0),
                                    stop=(t == T - 1),
                                    skip_group_check=True,
                                )
                    if pipe_c < NT1 and tl < 8 * CH:
                        g, k = tl // CH, tl % CH
                        if k == 0:
                            p1_ps = psum1.tile([128, PCOL], F32, tag="ps1")
                        p1_group_mm(p1_sq, p1_ps, g, k)

                    # n path first: tn/tanh/d run while the z-gate matmuls finish
                    tn = ewps.tile([128, D], F32, tag="tn")
                    nc.vector.tensor_add(tn[:], psn_acc[:], in_sb[:, t, :])
                    nt = ew_pool.tile([128, D], BF16, tag="nt")
                    nc.scalar.activation(nt[:], tn[:], AF.Tanh)
                    d = ew_pool.tile([128, D], BF16, tag="d")
                    nc.vector.tensor_sub(d[:], nt[:], h[:])
                    # z path: h_new = h + sigmoid(-tz)*(n - h)
                    tz = ewps.tile([128, D], F32, tag="tz")
                    nc.vector.tensor_add(tz[:], psz_acc[:], iz_sb[:, t, :])
                    zc = ew_pool.tile([128, D], BF16, tag="zc")
                    nc.scalar.activation(zc[:], tz[:], AF.Sigmoid, scale=-1.0)
                    d2 = ew_pool.tile([128, D], BF16, tag="d2")
                    nc.vector.tensor_mul(d2[:], zc[:], d[:])
                    hn = htb[:, tl, :]
                    nc.vector.tensor_add(hn, h[:], d2[:])
                    if pipe_c < NT1 and tl < 8 * CH and tl % CH == CH - 1:
                        p1_group_bias(p1_ps, pipe_c, tl // CH)
                    h = hn
                    d2_prev = d2

                nc.gpsimd.dma_start(HT[:, blk * BLK : (blk + 1) * BLK, :], htb[:])

    nc.compile()
    return nc


_CACHE: dict = {}


def _get_nc(T, BC):
    key = (T, BC)
    if key not in _CACHE:
        _CACHE[key] = build_nc(T, BC)
    return _CACHE[key]


def _in_maps(inputs, T, BC):
    f32 = np.float32
    wizT = np.ascontiguousarray(np.asarray(inputs["W_iz"], f32).T.astype(NPBF16))
    winT = np.ascontiguousarray(np.asarray(inputs["W_in"], f32).T.astype(NPBF16))
    whzT = np.ascontiguousarray(np.asarray(inputs["W_hz"], f32).T.astype(NPBF16))
    whnT = np.ascontiguousarray(np.asarray(inputs["W_hn"], f32).T.astype(NPBF16))
    biasz = np.ascontiguousarray(
        (np.asarray(inputs["b_iz"], f32) + np.asarray(inputs["b_hz"], f32)).reshape(CH, 128).T
    )
    biasn = np.ascontiguousarray(
        (np.asarray(inputs["b_in"], f32) + np.asarray(inputs["b_hn"], f32)).reshape(CH, 128).T
    )
    seq = np.asarray(inputs["seq"], f32)
    in_maps = []
    for c in range(NCORES):
        shard = seq[:, c * BC : (c + 1) * BC, :].reshape(T * BC, DIN)
        seqT = np.ascontiguousarray(shard.T.astype(NPBF16))
        in_maps.append(
            {
                "seqT": seqT,
                "wizT": wizT,
                "winT": winT,
                "whzT": whzT,
                "whnT": whnT,
                "biasz": biasz,
                "biasn": biasn,
            }
        )
    return in_maps


def _assemble(results, T, BC):
    out = np.empty((T, NCORES * BC, DH), np.float32)
    for c in range(NCORES):
        HT = np.asarray(results[c]["HT"], dtype=np.float32)  # [128, T, CH*BC]
        # HT[p, t, m*BC+b] -> out[t, b, m*128+p]
        Hc = HT.reshape(128, T, CH, BC).transpose(1, 3, 2, 0).reshape(T, BC, DH)
        out[:, c * BC : (c + 1) * BC, :] = Hc
    return out[None]


def kernel(seq, W_iz, b_iz, W_in, b_in, W_hz, b_hz, W_hn, b_hn):
    seq = np.asarray(seq)
    T, B, _ = seq.shape
    BC = B // NCORES
    nc = _get_nc(T, BC)
    in_maps = _in_maps(
        dict(seq=seq, W_iz=W_iz, b_iz=b_iz, W_in=W_in, b_in=b_in,
             W_hz=W_hz, b_hz=b_hz, W_hn=W_hn, b_hn=b_hn),
        T, BC,
    )
    res = bass_utils.run_bass_kernel_spmd(nc, in_maps, list(range(NCORES)))
    return _assemble(res.results, T, BC)


def traced_run(inputs):
    seq = np.asarray(inputs["seq"])
    T, B, _ = seq.shape
    BC = B // NCORES
    nc = _get_nc(T, BC)
    in_maps = _in_maps(inputs, T, BC)
    return bass_utils.run_bass_kernel_spmd(
        nc, in_maps, list(range(NCORES)), trace=True
    )
